# revision 53
# baseline (speedup 1.0000x reference)
"""Causal multi-head attention block (B=2, T=2048, D=1024, H=16) on 8 TRN2 cores.

Sharding: tensor-parallel over heads — each core owns 2 heads (128 cols of
w_attn's q/k/v blocks, 128 rows of w_proj) and produces a partial output
[B, T, D]; the host sums the 8 partials and adds the bias terms.

v5 layout (fp16 everywhere except the q/k projection, fp32 PSUM):
  - fp16 instead of bf16 (same PE/DVE rate, 8x the mantissa) keeps the
    base pipeline error at ~4e-4; the only deliberate precision spend is
    the q/k projection in fp8 e4m3 DoubleRow (pairs of d-tiles packed in
    the free dim, 2x PE rate; q/k feed softmax logits where ~1e-2 total
    error is tolerable). v / scores / AV / proj stay fp16.
  - k is stored zero-padded per head (kTz[h]: head h's rows + zeros) so
    score matmuls contract over the full 128 partitions with the
    two-head qT as moving operand: a 64-partition matmul measures ~2.4x
    slower per column on TRN2 hardware than a 128-partition one.
  - v is produced DIRECTLY in [t, hd] layout with x-stationary matmuls
    (stationary = x d-tile [128d, 128t], moving = w_v [128d, 128f]); the
    old vT + XBAR-transpose path (which stalled the PE ~11us) is gone.
    All four 128-t tiles of a chunk accumulate into one PSUM bank and
    are flipped into v_sb with a single strided DVE copy.
  - attention kt-loop is software-pipelined: scores/exp/mask of k-tile
    kt+1 are emitted BEFORE the AV matmuls of k-tile kt, so the
    scores->exp->AV round trip latency is hidden behind PE work instead
    of relying solely on filler items.
  - exp covers both heads in one ACT op (psum tile [128, 2, 512]); the
    causal mask is applied POST-exp as a {0,1} fp16 multiply on the
    diagonal 128-block of both heads in one cheap DVE op (4x DVE mode).
  - engine balance (measured on HW, ACT is the tightest): ACT = exp +
    q/k epilogues + one normalize staging copy; DVE = o-copies, masks,
    v epilogue, recip + normalize multiply; Pool = denominator
    broadcast + x DMA queue.
  - per-head normalize is emitted right after that head's last AV and
    stages av out of PSUM first, so the av bank frees immediately.
  - timing build (defer_tail) is fully rotated: batch-0 QKV of the NEXT
    pass and the last chunk's projections drain as fillers through this
    pass's attention, so the PE never sees a phase boundary.
  - PSUM budget (8 banks): scores 2x2 + av 2x1 + shared qkv/v/proj 2x1.
"""
import numpy as np

import concourse.bass as bass
import concourse.mybir as mybir
import concourse.tile as tile
from concourse import bacc
from concourse.bass import ts, ds
from concourse.bass_utils import run_bass_kernel_spmd

F32 = mybir.dt.float32
F16 = mybir.dt.float16
F8 = mybir.dt.float8e4
DR = mybir.MatmulPerfMode.DoubleRow

B, T, D = 2, 2048, 1024
H = 16
HD = D // H          # 64
N_CORES = 8
HPC = H // N_CORES   # heads per core = 2
CW = HPC * HD        # per-core head width = 128
TCH = 512            # q/t chunk width
NKT = T // 128       # 16 k-tiles per batch
NQC = T // TCH       # 4 q-chunks per batch
ND = D // 128        # 8 d-tiles
NT = B * T // 128    # 32 t-tiles total


def build_program(reps: int = 1, debug_out: bool = False):
    nc = bacc.Bacc("TRN2", target_bir_lowering=False, debug=False,
                   num_devices=N_CORES)

    xT = nc.dram_tensor("xT", [B, D, T], F16, kind="ExternalInput")
    x8T = nc.dram_tensor("x8T", [B, D, T], F8, kind="ExternalInput")
    wq = nc.dram_tensor("wq", [D, CW], F8, kind="ExternalInput")
    wk = nc.dram_tensor("wk", [D, CW], F8, kind="ExternalInput")
    wv = nc.dram_tensor("wv", [D, CW], F16, kind="ExternalInput")
    bq = nc.dram_tensor("bq", [CW], F32, kind="ExternalInput")   # pre-scaled /8
    bk = nc.dram_tensor("bk", [CW], F32, kind="ExternalInput")
    wp = nc.dram_tensor("wp", [CW, D], F16, kind="ExternalInput")
    # multiplicative causal mask for the diagonal 128-block, duplicated per
    # head: [128 k, HPC, 128 q], 1.0 where k <= q else 0.0
    mask = nc.dram_tensor("mask", [128, HPC, 128], F16, kind="ExternalInput")
    out = nc.dram_tensor("out", [B, T, D], F16, kind="ExternalOutput")
    if debug_out:
        dbg = {nm: nc.dram_tensor(f"dbg_{nm}", [128, B * T], F16,
                                  kind="ExternalOutput")
               for nm in ("qT", "kT", "a")}
        dbg["v2"] = nc.dram_tensor("dbg_v2", [128, NT, HPC, HD + 1],
                                   F16, kind="ExternalOutput")

    with tile.TileContext(nc) as tc:
        with (
            tc.tile_pool(name="const", bufs=1) as const,
            tc.tile_pool(name="persist", bufs=1) as persist,
            tc.tile_pool(name="xt", bufs=5) as xt_pool,
            tc.tile_pool(name="probs", bufs=6) as probs_pool,
            tc.tile_pool(name="norm", bufs=4) as norm_pool,
            tc.tile_pool(name="osb", bufs=8) as osb_pool,
        ):
            # ---- constants / persistent state ----
            wq_sb = const.tile([128, ND, CW], F8)
            wk_sb = const.tile([128, ND, CW], F8)
            wv_sb = const.tile([128, ND, CW], F16)
            wp_sb = const.tile([128, D], F16)
            bq_sb = const.tile([128, 1], F32)
            bk_sb = const.tile([128, 1], F32)
            mask_sb = const.tile([128, HPC, 128], F16)
            # consts on the sync queue; x streams on the gpsimd queue so the
            # steady-state x prefetch is not blocked behind out-writes.
            # wp/mask are not needed until attention starts — load them last.
            wq_r = wq.rearrange("(dt p) m -> p dt m", p=128)
            wk_r = wk.rearrange("(dt p) m -> p dt m", p=128)
            nc.sync.dma_start(wq_sb[:, 0:1, :], wq_r[:, 0:1, :])
            nc.sync.dma_start(wk_sb[:, 0:1, :], wk_r[:, 0:1, :])
            nc.sync.dma_start(wq_sb[:, 1:, :], wq_r[:, 1:, :])
            nc.sync.dma_start(wk_sb[:, 1:, :], wk_r[:, 1:, :])
            nc.sync.dma_start(wv_sb[:], wv.rearrange("(dt p) m -> p dt m", p=128))
            nc.sync.dma_start(bq_sb[:], bq[:, None])
            nc.sync.dma_start(bk_sb[:], bk[:, None])
            nc.sync.dma_start(mask_sb[:], mask[:, :, :])
            nc.sync.dma_start(wp_sb[:], wp[:, :])

            qT_sb = persist.tile([128, B * T], F8)   # [2h*64, (b,t)]
            # kT stored zero-padded per head: kTz[h] has head h's k in its
            # own 64 partition rows and ZEROS in the other head's rows, so
            # the score matmul contracts over the full 128 partitions (a
            # 64-partition matmul runs ~2.4x slower on the PE) with the
            # full two-head qT as the moving operand.
            # pair-slot 1 is ALL ZERO: the score DoubleRow matmul computes
            # k0^T q + 0^T q = k^T q exactly, at the measured ~3x fp8-DR
            # rate, with the moving q pair supplied as a stride-0 broadcast.
            kTz = [persist.tile([128, 2, B * T], F8, name=f"kTz{h}")
                   for h in range(HPC)]
            nc.vector.memset(kTz[0][HD:128, 0, :], 0.0)
            nc.vector.memset(kTz[1][0:HD, 0, :], 0.0)
            nc.vector.memset(kTz[0][:, 1, :], 0.0)
            nc.vector.memset(kTz[1][:, 1, :], 0.0)
            a_sb = persist.tile([128, B * T], F16)    # normalized attn out ^T
            # v per t-tile & head: [ones | v(64)] columns — the LEADING ones
            # column makes the AV matmul emit the softmax denominator in
            # psum partition 0, where reciprocal/broadcast read it directly.
            v_sb = persist.tile([128, NT, HPC, HD + 1], F16)
            nc.vector.memset(v_sb[:, :, :, 0], 1.0)

            def body(defer_tail=False, _=None):
                with (
                    tc.tile_pool(name="ps_s", bufs=2, space="PSUM") as ps_s,
                    tc.tile_pool(name="ps_av", bufs=2, space="PSUM") as ps_av,
                    tc.tile_pool(name="ps_x", bufs=2, space="PSUM") as ps_x,
                ):
                    def emit_x_dma(c, fine=False):
                        """stream x chunk c: fp8 copy (q/k DoubleRow gemms)
                        first, then the fp16 copy (v gemm)."""
                        xt8 = xt_pool.tile([128, ND, TCH], F8, name=f"xt8{c}",
                                           tag="xt8")
                        xt = xt_pool.tile([128, ND, TCH], F16, name=f"xt{c}",
                                          tag="xt")
                        b_i, qc = divmod(c, NQC)
                        x8src = x8T[b_i].rearrange("(dt p) t -> p dt t",
                                                   p=128)[:, :, ds(qc * TCH, TCH)]
                        xsrc = xT[b_i].rearrange("(dt p) t -> p dt t", p=128)[
                            :, :, ds(qc * TCH, TCH)]
                        gw = 2 if fine and c <= 1 else ND
                        for g in range(ND // gw):
                            nc.gpsimd.dma_start(xt8[:, ts(g, gw), :],
                                                x8src[:, ts(g, gw), :])
                        for g in range(ND // gw):
                            nc.gpsimd.dma_start(xt[:, ts(g, gw), :],
                                                xsrc[:, ts(g, gw), :])
                        return xt8, xt

                    def qk_epi(kind, dst_sb, acc, ccol):
                        # on ACT (Identity with per-partition bias + scale):
                        # keeps the elastic psum drains off the DVE queue,
                        # which carries the latency-critical mask multiplies
                        if kind == "q":
                            nc.scalar.activation(
                                dst_sb[:, ccol], acc[:],
                                mybir.ActivationFunctionType.Identity,
                                bias=bq_sb[:], scale=0.125)
                        else:
                            # k lands in the zero-padded per-head tensors;
                            # both writes are lane-aligned (no shift).
                            # One half per engine to balance ACT/DVE load.
                            nc.scalar.activation(
                                kTz[0][0:HD, 0, ccol], acc[0:HD, :],
                                mybir.ActivationFunctionType.Identity,
                                bias=bk_sb[0:HD])
                            nc.scalar.activation(
                                kTz[1][HD:128, 0, ccol], acc[HD:128, :],
                                mybir.ActivationFunctionType.Identity,
                                bias=bk_sb[HD:128])

                    def emit_qk_mms(c, xt8):
                        """q/k projection via fp8 DoubleRow (pairs of d-tiles
                        packed in the free dim) + epilogues for chunk c."""
                        ccol = ds(c * TCH, TCH)
                        for dst_sb, w_t, kind in ((qT_sb, wq_sb, "q"),
                                                  (None, wk_sb, "k")):
                            acc = ps_x.tile([128, TCH], F32, tag="x", name="acc")
                            for p in range(ND // 2):
                                nc.tensor.matmul(acc[:],
                                                 w_t[:, ts(p, 2), :],
                                                 xt8[:, ts(p, 2), :],
                                                 start=p == 0,
                                                 stop=p == ND // 2 - 1,
                                                 perf_mode=DR)
                            qk_epi(kind, dst_sb, acc, ccol)

                    def emit_v_mms(c, xt):
                        """v for chunk c directly in [t, hd] layout:
                        stationary x t-tile, moving w_v. All 4 t-tiles
                        accumulate into one PSUM bank; one strided DVE copy
                        flips into v_sb [t, h, 1+hd]."""
                        vacc = ps_x.tile([128, TCH], F32, tag="x", name="vacc")
                        for tt in range(TCH // 128):
                            for dt in range(ND):
                                nc.tensor.matmul(
                                    vacc[:, ts(tt, 128)],
                                    xt[:, dt, ts(tt, 128)],
                                    wv_sb[:, dt, :],
                                    start=dt == 0, stop=dt == ND - 1)
                        gt0 = c * (TCH // 128)
                        nc.vector.tensor_copy(
                            v_sb[:, gt0:gt0 + 4, :, 1:HD + 1],
                            vacc.rearrange("p (tt h d) -> p tt h d",
                                           tt=4, h=HPC))

                    def emit_proj(pb, tt, ec, i):
                        o_ps = ps_x.tile([128, TCH], F32, tag="x", name="o")
                        nc.tensor.matmul(
                            o_ps[:],
                            a_sb[:, ds(pb * T + tt * 128, 128)],
                            wp_sb[:, ts(ec, TCH)],
                            start=True, stop=True)
                        o_sb = osb_pool.tile([128, TCH], F16)
                        nc.vector.tensor_copy(o_sb[:], o_ps[:])
                        nc.sync.dma_start(
                            out[pb, ts(tt, 128), ts(ec, TCH)], o_sb[:])

                    # ---- unified PE filler queue (see v4) ----
                    fillq = []   # [(need_before_attn_chunk, fn)]

                    def qkv_items(c, xt8, xt):
                        """filler items for chunk c's q/k/v matmuls."""
                        items = []
                        ccol = ds(c * TCH, TCH)
                        for dst_sb, w_t, kind in ((qT_sb, wq_sb, "q"),
                                                  (None, wk_sb, "k")):
                            box = []

                            def h1(box=box, w_t=w_t, xt8=xt8):
                                acc = ps_x.tile([128, TCH], F32, tag="x",
                                                name="acc")
                                box.append(acc)
                                for p in range(ND // 4):
                                    nc.tensor.matmul(
                                        acc[:], w_t[:, ts(p, 2), :],
                                        xt8[:, ts(p, 2), :],
                                        start=p == 0, stop=False,
                                        perf_mode=DR)

                            def h2(box=box, w_t=w_t, xt8=xt8, kind=kind,
                                   dst_sb=dst_sb, ccol=ccol):
                                acc = box[0]
                                for p in range(ND // 4, ND // 2):
                                    nc.tensor.matmul(
                                        acc[:], w_t[:, ts(p, 2), :],
                                        xt8[:, ts(p, 2), :],
                                        start=False, stop=p == ND // 2 - 1,
                                        perf_mode=DR)
                                qk_epi(kind, dst_sb, acc, ccol)

                            items += [h1, h2]

                        vbox = []

                        def v_item(tt, c=c, xt=xt, vbox=vbox):
                            if tt == 0:
                                vacc = ps_x.tile([128, TCH], F32, tag="x",
                                                 name="vacc")
                                vbox.append(vacc)
                            vacc = vbox[0]
                            for dt in range(ND):
                                nc.tensor.matmul(
                                    vacc[:, ts(tt, 128)],
                                    xt[:, dt, ts(tt, 128)],
                                    wv_sb[:, dt, :],
                                    start=dt == 0, stop=dt == ND - 1)
                            if tt == 3:
                                gt0 = c * (TCH // 128)
                                nc.vector.tensor_copy(
                                    v_sb[:, gt0:gt0 + 4, :, 1:HD + 1],
                                    vacc.rearrange("p (tt h d) -> p tt h d",
                                                   tt=4, h=HPC))

                        items += [lambda tt=tt: v_item(tt) for tt in range(4)]
                        return items

                    # ---- phase 1a: QKV for batch 0 ----
                    # In the rotated steady-state body (defer_tail), batch
                    # 0's QKV for the NEXT pass drains as fillers through
                    # batch 1's attention; this pass's batch-0 attention
                    # reads qT/kT/v written by the previous pass.
                    if not defer_tail:
                        xts = {c: emit_x_dma(c, fine=True) for c in range(2)}
                        for c in range(NQC):
                            if c + 2 < NQC:
                                xts[c + 2] = emit_x_dma(c + 2)
                            xt8, xt = xts.pop(c)
                            emit_qk_mms(c, xt8)
                            emit_v_mms(c, xt)

                    # ---- attention; QKV for batch 1 + all projections are
                    # drained through the filler queue ----
                    if defer_tail:
                        fillq += [
                            (7, lambda t_=t_, e_=e_: emit_proj(
                                B - 1, t_, e_, t_ * 2 + e_))
                            for t_ in range((NQC - 1) * 4, NQC * 4)
                            for e_ in range(D // TCH)]
                    xts_n = {}
                    for b_i in range(B):
                        for qc in range(NQC):
                            g = b_i * NQC + qc   # global chunk index
                            if b_i == 0:
                                xt_c = emit_x_dma(4 + qc)
                                fillq += [(4 + qc, fn)
                                          for fn in qkv_items(4 + qc, *xt_c)]
                                if defer_tail and qc >= 2:
                                    # rotation: prefetch next pass's first
                                    # batch-0 x chunks early
                                    xts_n[qc - 2] = emit_x_dma(qc - 2)
                            elif defer_tail:
                                # rotation: next pass's batch-0 QKV drains
                                # as fillers through batch-1 attention
                                if qc < 2:
                                    xts_n[qc + 2] = emit_x_dma(qc + 2)
                                if qc == 0:
                                    for c_n in (0, 1):
                                        fillq += [(99, fn) for fn in
                                                  qkv_items(c_n, *xts_n[c_n])]
                                elif qc == 1:
                                    for c_n in (2, 3):
                                        fillq += [(99, fn) for fn in
                                                  qkv_items(c_n, *xts_n[c_n])]
                            # force-drain items this chunk depends on
                            rest = []
                            for need, fn in fillq:
                                if need <= g:
                                    fn()
                                else:
                                    rest.append((need, fn))
                            fillq = rest
                            qcol = ds(b_i * T + qc * TCH, TCH)
                            nkt = 4 * qc + 4   # causal: k-tiles 0..nkt-1
                            avs = [ps_av.tile([HD + 1, TCH], F32, tag="av",
                                              name=f"av{_h}")
                                   for _h in range(HPC)]

                            def emit_scores(kt):
                                """scores + exp + mask for k-tile kt; returns
                                (pp tile, fsl) ready for the AV matmuls."""
                                j = kt - 4 * qc   # >=0: straddles diagonal
                                f0 = max(j, 0) * 128
                                fsl = ds(f0, TCH - f0)
                                ktcol = ds(b_i * T + kt * 128, 128)
                                sp = ps_s.tile([128, HPC, TCH], F32,
                                               tag="sp", name="sp")
                                pp = probs_pool.tile(
                                    [128, HPC, TCH], F16,
                                    tag="pp", name="pp")
                                qmv = qT_sb[:, qcol][:, fsl]
                                qmv = qmv.rearrange(
                                    "p (o w) -> p o w", o=1).broadcast_to(
                                    [128, 2, TCH - f0])
                                for h in range(HPC):
                                    nc.tensor.matmul(
                                        sp[:, h, fsl],
                                        kTz[h][:, :, ktcol],
                                        qmv,
                                        start=True, stop=True,
                                        perf_mode=DR)
                                # one exp covers both heads (fewer ACT ops;
                                # a per-head split measured 8us slower, and
                                # Schraudolph exp-on-DVE measured 6us slower
                                # at a 1/3 fraction: both queues are tight)
                                if f0 == 0:
                                    nc.scalar.activation(
                                        pp.rearrange("p h w -> p (h w)"),
                                        sp.rearrange("p h w -> p (h w)"),
                                        mybir.ActivationFunctionType.Exp)
                                elif f0 == 0:
                                    nc.scalar.activation(
                                        pp.rearrange("p h w -> p (h w)"),
                                        sp.rearrange("p h w -> p (h w)"),
                                        mybir.ActivationFunctionType.Exp)
                                else:
                                    nc.scalar.activation(
                                        pp[:, :, fsl], sp[:, :, fsl],
                                        mybir.ActivationFunctionType.Exp)
                                if j >= 0:
                                    # multiplicative causal mask on the
                                    # diagonal 128-block of both heads
                                    dsl = ds(f0, 128)
                                    nc.vector.tensor_tensor(
                                        pp[:, :, dsl], pp[:, :, dsl],
                                        mask_sb[:, :, :],
                                        mybir.AluOpType.mult)
                                return pp, fsl

                            def emit_norm(h):
                                """normalize rows 1..64 by denominator row 0;
                                emitted immediately after head h's last AV.
                                The av psum bank is freed by a single staging
                                copy (one ACT/DVE op) so the next chunk's AV
                                never waits on the recip/bcast/mult chain."""
                                hp = ds(h * HD, HD)
                                av_ps = avs[h]
                                av_s = norm_pool.tile([HD + 1, TCH], F32,
                                                      tag=f"avs{h}")
                                if h == 0:
                                    nc.scalar.copy(av_s[:], av_ps[:])
                                else:
                                    nc.vector.tensor_copy(av_s[:], av_ps[:])
                                r0_sb = norm_pool.tile([1, TCH], F32,
                                                       tag="r0")
                                nc.vector.reciprocal_approx_fast(
                                    r0_sb[:], av_ps[0:1, :])
                                bc_sb = norm_pool.tile([HD + 1, TCH], F32,
                                                       tag="bc")
                                nc.gpsimd.partition_broadcast(
                                    bc_sb[:], r0_sb[:])
                                at_sb = norm_pool.tile([HD + 1, TCH], F16,
                                                       tag="at")
                                # row 0 computes junk (denom*recip) that no
                                # reader touches; base must be 0 (aligned)
                                nc.vector.tensor_tensor(
                                    at_sb[0:HD + 1, :], av_s[0:HD + 1, :],
                                    bc_sb[0:HD + 1, :],
                                    mybir.AluOpType.mult)
                                # partition-shift into stacked-head layout
                                nc.sync.dma_start(a_sb[hp, qcol],
                                                  at_sb[1:HD + 1, :])

                            def emit_av(kt, pp, fsl):
                                for h in range(HPC):
                                    nc.tensor.matmul(
                                        avs[h][:, fsl],
                                        v_sb[:, b_i * NKT + kt, h, :],
                                        pp[:, h, fsl],
                                        start=kt == 0, stop=kt == nkt - 1)
                                    if kt == nkt - 1:
                                        emit_norm(h)

                            # software pipeline: AV lags scores by one k-tile
                            pend = None
                            for kt in range(nkt):
                                cur = (kt, *emit_scores(kt))
                                if pend is not None:
                                    npop = 2 if len(fillq) > 10 else 1
                                    for _p in range(min(npop, len(fillq))):
                                        fillq.pop(0)[1]()
                                    emit_av(*pend)
                                pend = cur
                            if fillq:
                                fillq.pop(0)[1]()
                            emit_av(*pend)

                            items = [(b_i, t_, e_, t_ * 2 + e_)
                                     for t_ in range(qc * 4, qc * 4 + 4)
                                     for e_ in range(D // TCH)]
                            last = (b_i == B - 1 and qc == NQC - 1)
                            if not last:
                                fillq += [(99, lambda a=a: emit_proj(*a))
                                          for a in items]
                            elif not defer_tail:
                                for a in items:
                                    emit_proj(*a)
                            if last:
                                for _, fn in fillq:
                                    fn()
                                fillq = []

            if reps < 0:
                # static unroll of the rotated body, for TimelineSim only
                # (the sim cannot follow dynamic loops)
                for _r in range(-reps):
                    body(defer_tail=True)
            elif reps == 1:
                body(defer_tail=False)
                if debug_out:
                    nc.sync.dma_start(dbg["qT"][:, :], qT_sb[:])
                    nc.sync.dma_start(dbg["kT"][:, :], kTz[0][:, 0, :])
                    nc.sync.dma_start(dbg["a"][:, :], a_sb[:])
                    nc.sync.dma_start(dbg["v2"][:, :, :, :], v_sb[:])
            else:
                with tc.For_i(0, reps, 1) as _i:
                    body(defer_tail=True, _=_i)

    nc.compile()
    return nc


def make_mask() -> np.ndarray:
    """Multiplicative causal mask for the 128x128 diagonal block, duplicated
    per head: keeps (p <= f)."""
    p = np.arange(128)[:, None]
    f = np.arange(128)[None, :]
    m = (p <= f).astype(np.float16)
    return np.broadcast_to(m[:, None, :], (128, HPC, 128)).copy()


def _e4m3(a):
    import ml_dtypes
    return np.clip(a, -240.0, 240.0).astype(ml_dtypes.float8_e4m3fn)


def make_in_maps(x, w_attn, b_attn, w_proj):
    f16 = np.float16
    xT = np.ascontiguousarray(np.transpose(x, (0, 2, 1))).astype(f16)
    x8T = _e4m3(xT.astype(np.float32))
    mask = make_mask()
    in_maps = []
    for c in range(N_CORES):
        cs = slice(CW * c, CW * (c + 1))
        in_maps.append({
            "xT": xT,
            "x8T": x8T,
            "wq": _e4m3(np.ascontiguousarray(w_attn[:, 0 * D:1 * D][:, cs])),
            "wk": _e4m3(np.ascontiguousarray(w_attn[:, 1 * D:2 * D][:, cs])),
            "wv": np.ascontiguousarray(w_attn[:, 2 * D:3 * D][:, cs]).astype(f16),
            "bq": np.ascontiguousarray(b_attn[0 * D:1 * D][cs]) * 0.125,
            "bk": np.ascontiguousarray(b_attn[1 * D:2 * D][cs]),
            "wp": np.ascontiguousarray(w_proj[cs, :]).astype(f16),
            "mask": mask,
        })
    return in_maps


def host_bias(b_attn, b_proj, w_proj):
    # v-bias propagates exactly through softmax (rows sum to 1) and the linear
    # projection: out += b_v @ w_proj + b_proj
    return b_proj.astype(np.float32) + b_attn[2 * D:3 * D].astype(np.float32) @ w_proj.astype(np.float32)


_NC_CACHE = {}


def get_program(reps: int = 1, debug_out: bool = False):
    key = (reps, debug_out)
    if key not in _NC_CACHE:
        _NC_CACHE[key] = build_program(reps, debug_out)
    return _NC_CACHE[key]


def kernel(x, w_attn, b_attn, w_proj, b_proj):
    x = np.asarray(x, np.float32)
    w_attn = np.asarray(w_attn, np.float32)
    b_attn = np.asarray(b_attn, np.float32)
    w_proj = np.asarray(w_proj, np.float32)
    b_proj = np.asarray(b_proj, np.float32)

    nc = get_program()
    in_maps = make_in_maps(x, w_attn, b_attn, w_proj)
    res = run_bass_kernel_spmd(nc, in_maps, core_ids=list(range(N_CORES)))
    acc = np.zeros((B, T, D), np.float64)
    for r in res.results:
        acc += r["out"].astype(np.float64)
    acc += host_bias(b_attn, b_proj, w_proj).astype(np.float64)
    return acc.astype(np.float32)


# revision 54
# speedup vs baseline: 1.0422x; 1.0422x over previous
"""Causal multi-head attention block (B=2, T=2048, D=1024, H=16) on 8 TRN2 cores.

Sharding: tensor-parallel over heads — each core owns 2 heads (128 cols of
w_attn's q/k/v blocks, 128 rows of w_proj) and produces a partial output
[B, T, D]; the host sums the 8 partials and adds the bias terms.

v5 layout (fp16 everywhere except the q/k projection, fp32 PSUM):
  - fp16 instead of bf16 (same PE/DVE rate, 8x the mantissa) keeps the
    base pipeline error at ~4e-4; the only deliberate precision spend is
    the q/k projection in fp8 e4m3 DoubleRow (pairs of d-tiles packed in
    the free dim, 2x PE rate; q/k feed softmax logits where ~1e-2 total
    error is tolerable). v / scores / AV / proj stay fp16.
  - k is stored zero-padded per head (kTz[h]: head h's rows + zeros) so
    score matmuls contract over the full 128 partitions with the
    two-head qT as moving operand: a 64-partition matmul measures ~2.4x
    slower per column on TRN2 hardware than a 128-partition one.
  - v is produced DIRECTLY in [t, hd] layout with x-stationary matmuls
    (stationary = x d-tile [128d, 128t], moving = w_v [128d, 128f]); the
    old vT + XBAR-transpose path (which stalled the PE ~11us) is gone.
    All four 128-t tiles of a chunk accumulate into one PSUM bank and
    are flipped into v_sb with a single strided DVE copy.
  - attention kt-loop is software-pipelined: scores/exp/mask of k-tile
    kt+1 are emitted BEFORE the AV matmuls of k-tile kt, so the
    scores->exp->AV round trip latency is hidden behind PE work instead
    of relying solely on filler items.
  - exp covers both heads in one ACT op (psum tile [128, 2, 512]); the
    causal mask is applied POST-exp as a {0,1} fp16 multiply on the
    diagonal 128-block of both heads in one cheap DVE op (4x DVE mode).
  - engine balance (measured on HW, ACT is the tightest): ACT = exp +
    q/k epilogues + one normalize staging copy; DVE = o-copies, masks,
    v epilogue, recip + normalize multiply; Pool = denominator
    broadcast + x DMA queue.
  - per-head normalize is emitted right after that head's last AV and
    stages av out of PSUM first, so the av bank frees immediately.
  - timing build (defer_tail) is fully rotated: batch-0 QKV of the NEXT
    pass and the last chunk's projections drain as fillers through this
    pass's attention, so the PE never sees a phase boundary.
  - PSUM budget (8 banks): scores 2x2 + av 2x1 + shared qkv/v/proj 2x1.
"""
import numpy as np

import concourse.bass as bass
import concourse.mybir as mybir
import concourse.tile as tile
from concourse import bacc
from concourse.bass import ts, ds
from concourse.bass_utils import run_bass_kernel_spmd

F32 = mybir.dt.float32
F16 = mybir.dt.float16
F8 = mybir.dt.float8e4
DR = mybir.MatmulPerfMode.DoubleRow

B, T, D = 2, 2048, 1024
H = 16
HD = D // H          # 64
N_CORES = 8
HPC = H // N_CORES   # heads per core = 2
CW = HPC * HD        # per-core head width = 128
TCH = 512            # q/t chunk width
NKT = T // 128       # 16 k-tiles per batch
NQC = T // TCH       # 4 q-chunks per batch
ND = D // 128        # 8 d-tiles
NT = B * T // 128    # 32 t-tiles total


def build_program(reps: int = 1, debug_out: bool = False):
    nc = bacc.Bacc("TRN2", target_bir_lowering=False, debug=False,
                   num_devices=N_CORES)

    xT = nc.dram_tensor("xT", [B, D, T], F16, kind="ExternalInput")
    x8T = nc.dram_tensor("x8T", [B, D, T], F8, kind="ExternalInput")
    wq = nc.dram_tensor("wq", [D, CW], F8, kind="ExternalInput")
    wk = nc.dram_tensor("wk", [D, CW], F8, kind="ExternalInput")
    wv = nc.dram_tensor("wv", [D, CW], F16, kind="ExternalInput")
    bq = nc.dram_tensor("bq", [CW], F32, kind="ExternalInput")   # pre-scaled /8
    bk = nc.dram_tensor("bk", [CW], F32, kind="ExternalInput")
    wp = nc.dram_tensor("wp", [CW, D], F16, kind="ExternalInput")
    # multiplicative causal mask for the diagonal 128-block, duplicated per
    # head: [128 k, HPC, 128 q], 1.0 where k <= q else 0.0
    mask = nc.dram_tensor("mask", [128, HPC, 128], F16, kind="ExternalInput")
    out = nc.dram_tensor("out", [B, T, D], F16, kind="ExternalOutput")
    if debug_out:
        dbg = {nm: nc.dram_tensor(f"dbg_{nm}", [128, B * T], F16,
                                  kind="ExternalOutput")
               for nm in ("qT", "kT", "a")}
        dbg["v2"] = nc.dram_tensor("dbg_v2", [128, NT, HPC, HD + 1],
                                   F16, kind="ExternalOutput")

    with tile.TileContext(nc) as tc:
        with (
            tc.tile_pool(name="const", bufs=1) as const,
            tc.tile_pool(name="persist", bufs=1) as persist,
            tc.tile_pool(name="xt", bufs=5) as xt_pool,
            tc.tile_pool(name="probs", bufs=6) as probs_pool,
            tc.tile_pool(name="norm", bufs=4) as norm_pool,
            tc.tile_pool(name="osb", bufs=8) as osb_pool,
        ):
            # ---- constants / persistent state ----
            wq_sb = const.tile([128, ND, CW], F8)
            wk_sb = const.tile([128, ND, CW], F8)
            wv_sb = const.tile([128, ND, CW], F16)
            wp_sb = const.tile([128, D], F16)
            bq_sb = const.tile([128, 1], F32)
            bk_sb = const.tile([128, 1], F32)
            mask_sb = const.tile([128, HPC, 128], F16)
            # consts on the sync queue; x streams on the gpsimd queue so the
            # steady-state x prefetch is not blocked behind out-writes.
            # wp/mask are not needed until attention starts — load them last.
            wq_r = wq.rearrange("(dt p) m -> p dt m", p=128)
            wk_r = wk.rearrange("(dt p) m -> p dt m", p=128)
            nc.sync.dma_start(wq_sb[:, 0:1, :], wq_r[:, 0:1, :])
            nc.sync.dma_start(wk_sb[:, 0:1, :], wk_r[:, 0:1, :])
            nc.sync.dma_start(wq_sb[:, 1:, :], wq_r[:, 1:, :])
            nc.sync.dma_start(wk_sb[:, 1:, :], wk_r[:, 1:, :])
            nc.sync.dma_start(wv_sb[:], wv.rearrange("(dt p) m -> p dt m", p=128))
            nc.sync.dma_start(bq_sb[:], bq[:, None])
            nc.sync.dma_start(bk_sb[:], bk[:, None])
            nc.sync.dma_start(mask_sb[:], mask[:, :, :])
            nc.sync.dma_start(wp_sb[:], wp[:, :])

            qT_sb = persist.tile([128, B * T], F16)   # [2h*64, (b,t)]
            # kT stored zero-padded per head: kTz[h] has head h's k in its
            # own 64 partition rows and ZEROS in the other head's rows, so
            # the score matmul contracts over the full 128 partitions (a
            # 64-partition matmul runs ~2.4x slower on the PE) with the
            # full two-head qT as the moving operand.
            kTz = [persist.tile([128, B * T], F16, name=f"kTz{h}")
                   for h in range(HPC)]
            nc.vector.memset(kTz[0][HD:128, :], 0.0)
            nc.vector.memset(kTz[1][0:HD, :], 0.0)
            a_sb = persist.tile([128, B * T], F16)    # normalized attn out ^T
            # v per t-tile & head: [ones | v(64)] columns — the LEADING ones
            # column makes the AV matmul emit the softmax denominator in
            # psum partition 0, where reciprocal/broadcast read it directly.
            v_sb = persist.tile([128, NT, HPC, HD + 1], F16)
            nc.vector.memset(v_sb[:, :, :, 0], 1.0)

            def body(defer_tail=False, _=None):
                with (
                    tc.tile_pool(name="ps_s", bufs=2, space="PSUM") as ps_s,
                    tc.tile_pool(name="ps_av", bufs=2, space="PSUM") as ps_av,
                    tc.tile_pool(name="ps_x", bufs=2, space="PSUM") as ps_x,
                ):
                    def emit_x_dma(c, fine=False):
                        """stream x chunk c: fp8 copy (q/k DoubleRow gemms)
                        first, then the fp16 copy (v gemm)."""
                        xt8 = xt_pool.tile([128, ND, TCH], F8, name=f"xt8{c}",
                                           tag="xt8")
                        xt = xt_pool.tile([128, ND, TCH], F16, name=f"xt{c}",
                                          tag="xt")
                        b_i, qc = divmod(c, NQC)
                        x8src = x8T[b_i].rearrange("(dt p) t -> p dt t",
                                                   p=128)[:, :, ds(qc * TCH, TCH)]
                        xsrc = xT[b_i].rearrange("(dt p) t -> p dt t", p=128)[
                            :, :, ds(qc * TCH, TCH)]
                        gw = 2 if fine and c <= 1 else ND
                        for g in range(ND // gw):
                            nc.gpsimd.dma_start(xt8[:, ts(g, gw), :],
                                                x8src[:, ts(g, gw), :])
                        for g in range(ND // gw):
                            nc.gpsimd.dma_start(xt[:, ts(g, gw), :],
                                                xsrc[:, ts(g, gw), :])
                        return xt8, xt

                    def qk_epi(kind, dst_sb, acc, ccol):
                        # on ACT (Identity with per-partition bias + scale):
                        # keeps the elastic psum drains off the DVE queue,
                        # which carries the latency-critical mask multiplies
                        if kind == "q":
                            nc.scalar.activation(
                                dst_sb[:, ccol], acc[:],
                                mybir.ActivationFunctionType.Identity,
                                bias=bq_sb[:], scale=0.125)
                        else:
                            # k lands in the zero-padded per-head tensors;
                            # both writes are lane-aligned (no shift).
                            # One half per engine to balance ACT/DVE load.
                            nc.scalar.activation(
                                kTz[0][0:HD, ccol], acc[0:HD, :],
                                mybir.ActivationFunctionType.Identity,
                                bias=bk_sb[0:HD])
                            nc.scalar.activation(
                                kTz[1][HD:128, ccol], acc[HD:128, :],
                                mybir.ActivationFunctionType.Identity,
                                bias=bk_sb[HD:128])

                    def emit_qk_mms(c, xt8):
                        """q/k projection via fp8 DoubleRow (pairs of d-tiles
                        packed in the free dim) + epilogues for chunk c."""
                        ccol = ds(c * TCH, TCH)
                        for dst_sb, w_t, kind in ((qT_sb, wq_sb, "q"),
                                                  (None, wk_sb, "k")):
                            acc = ps_x.tile([128, TCH], F32, tag="x", name="acc")
                            for p in range(ND // 2):
                                nc.tensor.matmul(acc[:],
                                                 w_t[:, ts(p, 2), :],
                                                 xt8[:, ts(p, 2), :],
                                                 start=p == 0,
                                                 stop=p == ND // 2 - 1,
                                                 perf_mode=DR)
                            qk_epi(kind, dst_sb, acc, ccol)

                    def emit_v_mms(c, xt):
                        """v for chunk c directly in [t, hd] layout:
                        stationary x t-tile, moving w_v. All 4 t-tiles
                        accumulate into one PSUM bank; one strided DVE copy
                        flips into v_sb [t, h, 1+hd]."""
                        vacc = ps_x.tile([128, TCH], F32, tag="x", name="vacc")
                        for tt in range(TCH // 128):
                            for dt in range(ND):
                                nc.tensor.matmul(
                                    vacc[:, ts(tt, 128)],
                                    xt[:, dt, ts(tt, 128)],
                                    wv_sb[:, dt, :],
                                    start=dt == 0, stop=dt == ND - 1)
                        gt0 = c * (TCH // 128)
                        nc.vector.tensor_copy(
                            v_sb[:, gt0:gt0 + 4, :, 1:HD + 1],
                            vacc.rearrange("p (tt h d) -> p tt h d",
                                           tt=4, h=HPC))

                    def emit_proj(pb, tt, ec, i):
                        o_ps = ps_x.tile([128, TCH], F32, tag="x", name="o")
                        nc.tensor.matmul(
                            o_ps[:],
                            a_sb[:, ds(pb * T + tt * 128, 128)],
                            wp_sb[:, ts(ec, TCH)],
                            start=True, stop=True)
                        o_sb = osb_pool.tile([128, TCH], F16)
                        nc.vector.tensor_copy(o_sb[:], o_ps[:])
                        nc.sync.dma_start(
                            out[pb, ts(tt, 128), ts(ec, TCH)], o_sb[:])

                    # ---- unified PE filler queue (see v4) ----
                    fillq = []   # [(need_before_attn_chunk, fn)]

                    def qkv_items(c, xt8, xt):
                        """filler items for chunk c's q/k/v matmuls."""
                        items = []
                        ccol = ds(c * TCH, TCH)
                        for dst_sb, w_t, kind in ((qT_sb, wq_sb, "q"),
                                                  (None, wk_sb, "k")):
                            box = []

                            def h1(box=box, w_t=w_t, xt8=xt8):
                                acc = ps_x.tile([128, TCH], F32, tag="x",
                                                name="acc")
                                box.append(acc)
                                for p in range(ND // 4):
                                    nc.tensor.matmul(
                                        acc[:], w_t[:, ts(p, 2), :],
                                        xt8[:, ts(p, 2), :],
                                        start=p == 0, stop=False,
                                        perf_mode=DR)

                            def h2(box=box, w_t=w_t, xt8=xt8, kind=kind,
                                   dst_sb=dst_sb, ccol=ccol):
                                acc = box[0]
                                for p in range(ND // 4, ND // 2):
                                    nc.tensor.matmul(
                                        acc[:], w_t[:, ts(p, 2), :],
                                        xt8[:, ts(p, 2), :],
                                        start=False, stop=p == ND // 2 - 1,
                                        perf_mode=DR)
                                qk_epi(kind, dst_sb, acc, ccol)

                            items += [h1, h2]

                        vbox = []

                        def v_item(tt, c=c, xt=xt, vbox=vbox):
                            if tt == 0:
                                vacc = ps_x.tile([128, TCH], F32, tag="x",
                                                 name="vacc")
                                vbox.append(vacc)
                            vacc = vbox[0]
                            for dt in range(ND):
                                nc.tensor.matmul(
                                    vacc[:, ts(tt, 128)],
                                    xt[:, dt, ts(tt, 128)],
                                    wv_sb[:, dt, :],
                                    start=dt == 0, stop=dt == ND - 1)
                            if tt == 3:
                                gt0 = c * (TCH // 128)
                                nc.vector.tensor_copy(
                                    v_sb[:, gt0:gt0 + 4, :, 1:HD + 1],
                                    vacc.rearrange("p (tt h d) -> p tt h d",
                                                   tt=4, h=HPC))

                        items += [lambda tt=tt: v_item(tt) for tt in range(4)]
                        return items

                    # ---- phase 1a: QKV for batch 0 ----
                    # In the rotated steady-state body (defer_tail), batch
                    # 0's QKV for the NEXT pass drains as fillers through
                    # batch 1's attention; this pass's batch-0 attention
                    # reads qT/kT/v written by the previous pass.
                    if not defer_tail:
                        xts = {c: emit_x_dma(c, fine=True) for c in range(2)}
                        for c in range(NQC):
                            if c + 2 < NQC:
                                xts[c + 2] = emit_x_dma(c + 2)
                            xt8, xt = xts.pop(c)
                            emit_qk_mms(c, xt8)
                            emit_v_mms(c, xt)

                    # ---- attention; QKV for batch 1 + all projections are
                    # drained through the filler queue ----
                    if defer_tail:
                        fillq += [
                            (7, lambda t_=t_, e_=e_: emit_proj(
                                B - 1, t_, e_, t_ * 2 + e_))
                            for t_ in range((NQC - 1) * 4, NQC * 4)
                            for e_ in range(D // TCH)]
                    xts_n = {}
                    for b_i in range(B):
                        for qc in range(NQC):
                            g = b_i * NQC + qc   # global chunk index
                            if b_i == 0:
                                xt_c = emit_x_dma(4 + qc)
                                fillq += [(4 + qc, fn)
                                          for fn in qkv_items(4 + qc, *xt_c)]
                                if defer_tail and qc >= 2:
                                    # rotation: prefetch next pass's first
                                    # batch-0 x chunks early
                                    xts_n[qc - 2] = emit_x_dma(qc - 2)
                            elif defer_tail:
                                # rotation: next pass's batch-0 QKV drains
                                # as fillers through batch-1 attention
                                if qc < 2:
                                    xts_n[qc + 2] = emit_x_dma(qc + 2)
                                if qc == 0:
                                    for c_n in (0, 1):
                                        fillq += [(99, fn) for fn in
                                                  qkv_items(c_n, *xts_n[c_n])]
                                elif qc == 1:
                                    for c_n in (2, 3):
                                        fillq += [(99, fn) for fn in
                                                  qkv_items(c_n, *xts_n[c_n])]
                            # force-drain items this chunk depends on
                            rest = []
                            for need, fn in fillq:
                                if need <= g:
                                    fn()
                                else:
                                    rest.append((need, fn))
                            fillq = rest
                            qcol = ds(b_i * T + qc * TCH, TCH)
                            nkt = 4 * qc + 4   # causal: k-tiles 0..nkt-1
                            avs = [ps_av.tile([HD + 1, TCH], F32, tag="av",
                                              name=f"av{_h}")
                                   for _h in range(HPC)]

                            def emit_scores(kt):
                                """scores + exp + mask for k-tile kt; returns
                                (pp tile, fsl) ready for the AV matmuls."""
                                j = kt - 4 * qc   # >=0: straddles diagonal
                                f0 = max(j, 0) * 128
                                fsl = ds(f0, TCH - f0)
                                ktcol = ds(b_i * T + kt * 128, 128)
                                sp = ps_s.tile([128, HPC, TCH], F32,
                                               tag="sp", name="sp")
                                pp = probs_pool.tile(
                                    [128, HPC, TCH], F16,
                                    tag="pp", name="pp")
                                for h in range(HPC):
                                    nc.tensor.matmul(
                                        sp[:, h, fsl],
                                        kTz[h][:, ktcol],
                                        qT_sb[:, qcol][:, fsl],
                                        start=True, stop=True)
                                # one exp covers both heads (fewer ACT ops;
                                # a per-head split measured 8us slower, and
                                # Schraudolph exp-on-DVE measured 6us slower
                                # at a 1/3 fraction: both queues are tight)
                                if f0 == 0:
                                    nc.scalar.activation(
                                        pp.rearrange("p h w -> p (h w)"),
                                        sp.rearrange("p h w -> p (h w)"),
                                        mybir.ActivationFunctionType.Exp)
                                elif f0 == 0:
                                    nc.scalar.activation(
                                        pp.rearrange("p h w -> p (h w)"),
                                        sp.rearrange("p h w -> p (h w)"),
                                        mybir.ActivationFunctionType.Exp)
                                else:
                                    nc.scalar.activation(
                                        pp[:, :, fsl], sp[:, :, fsl],
                                        mybir.ActivationFunctionType.Exp)
                                if j >= 0:
                                    # multiplicative causal mask on the
                                    # diagonal 128-block of both heads
                                    dsl = ds(f0, 128)
                                    nc.vector.tensor_tensor(
                                        pp[:, :, dsl], pp[:, :, dsl],
                                        mask_sb[:, :, :],
                                        mybir.AluOpType.mult)
                                return pp, fsl

                            def emit_norm(h):
                                """normalize rows 1..64 by denominator row 0;
                                emitted immediately after head h's last AV.
                                The av psum bank is freed by a single staging
                                copy (one ACT/DVE op) so the next chunk's AV
                                never waits on the recip/bcast/mult chain."""
                                hp = ds(h * HD, HD)
                                av_ps = avs[h]
                                av_s = norm_pool.tile([HD + 1, TCH], F32,
                                                      tag=f"avs{h}")
                                if h == 0:
                                    nc.scalar.copy(av_s[:], av_ps[:])
                                else:
                                    nc.vector.tensor_copy(av_s[:], av_ps[:])
                                r0_sb = norm_pool.tile([1, TCH], F32,
                                                       tag="r0")
                                nc.vector.reciprocal_approx_fast(
                                    r0_sb[:], av_ps[0:1, :])
                                bc_sb = norm_pool.tile([HD + 1, TCH], F32,
                                                       tag="bc")
                                nc.gpsimd.partition_broadcast(
                                    bc_sb[:], r0_sb[:])
                                at_sb = norm_pool.tile([HD + 1, TCH], F16,
                                                       tag="at")
                                # row 0 computes junk (denom*recip) that no
                                # reader touches; base must be 0 (aligned)
                                nc.vector.tensor_tensor(
                                    at_sb[0:HD + 1, :], av_s[0:HD + 1, :],
                                    bc_sb[0:HD + 1, :],
                                    mybir.AluOpType.mult)
                                # partition-shift into stacked-head layout
                                nc.sync.dma_start(a_sb[hp, qcol],
                                                  at_sb[1:HD + 1, :])

                            def emit_av(kt, pp, fsl):
                                for h in range(HPC):
                                    nc.tensor.matmul(
                                        avs[h][:, fsl],
                                        v_sb[:, b_i * NKT + kt, h, :],
                                        pp[:, h, fsl],
                                        start=kt == 0, stop=kt == nkt - 1)
                                    if kt == nkt - 1:
                                        emit_norm(h)

                            # software pipeline: AV lags scores by one k-tile
                            pend = None
                            for kt in range(nkt):
                                cur = (kt, *emit_scores(kt))
                                if pend is not None:
                                    npop = 2 if len(fillq) > 10 else 1
                                    for _p in range(min(npop, len(fillq))):
                                        fillq.pop(0)[1]()
                                    emit_av(*pend)
                                pend = cur
                            if fillq:
                                fillq.pop(0)[1]()
                            emit_av(*pend)

                            items = [(b_i, t_, e_, t_ * 2 + e_)
                                     for t_ in range(qc * 4, qc * 4 + 4)
                                     for e_ in range(D // TCH)]
                            last = (b_i == B - 1 and qc == NQC - 1)
                            if not last:
                                fillq += [(99, lambda a=a: emit_proj(*a))
                                          for a in items]
                            elif not defer_tail:
                                for a in items:
                                    emit_proj(*a)
                            if last:
                                for _, fn in fillq:
                                    fn()
                                fillq = []

            if reps < 0:
                # static unroll of the rotated body, for TimelineSim only
                # (the sim cannot follow dynamic loops)
                for _r in range(-reps):
                    body(defer_tail=True)
            elif reps == 1:
                body(defer_tail=False)
                if debug_out:
                    nc.sync.dma_start(dbg["qT"][:, :], qT_sb[:])
                    nc.sync.dma_start(dbg["kT"][:, :], kTz[0][:])
                    nc.sync.dma_start(dbg["a"][:, :], a_sb[:])
                    nc.sync.dma_start(dbg["v2"][:, :, :, :], v_sb[:])
            else:
                with tc.For_i(0, reps, 1) as _i:
                    body(defer_tail=True, _=_i)

    nc.compile()
    return nc


def make_mask() -> np.ndarray:
    """Multiplicative causal mask for the 128x128 diagonal block, duplicated
    per head: keeps (p <= f)."""
    p = np.arange(128)[:, None]
    f = np.arange(128)[None, :]
    m = (p <= f).astype(np.float16)
    return np.broadcast_to(m[:, None, :], (128, HPC, 128)).copy()


def _e4m3(a):
    import ml_dtypes
    return np.clip(a, -240.0, 240.0).astype(ml_dtypes.float8_e4m3fn)


def make_in_maps(x, w_attn, b_attn, w_proj):
    f16 = np.float16
    xT = np.ascontiguousarray(np.transpose(x, (0, 2, 1))).astype(f16)
    x8T = _e4m3(xT.astype(np.float32))
    mask = make_mask()
    in_maps = []
    for c in range(N_CORES):
        cs = slice(CW * c, CW * (c + 1))
        in_maps.append({
            "xT": xT,
            "x8T": x8T,
            "wq": _e4m3(np.ascontiguousarray(w_attn[:, 0 * D:1 * D][:, cs])),
            "wk": _e4m3(np.ascontiguousarray(w_attn[:, 1 * D:2 * D][:, cs])),
            "wv": np.ascontiguousarray(w_attn[:, 2 * D:3 * D][:, cs]).astype(f16),
            "bq": np.ascontiguousarray(b_attn[0 * D:1 * D][cs]) * 0.125,
            "bk": np.ascontiguousarray(b_attn[1 * D:2 * D][cs]),
            "wp": np.ascontiguousarray(w_proj[cs, :]).astype(f16),
            "mask": mask,
        })
    return in_maps


def host_bias(b_attn, b_proj, w_proj):
    # v-bias propagates exactly through softmax (rows sum to 1) and the linear
    # projection: out += b_v @ w_proj + b_proj
    return b_proj.astype(np.float32) + b_attn[2 * D:3 * D].astype(np.float32) @ w_proj.astype(np.float32)


_NC_CACHE = {}


def get_program(reps: int = 1, debug_out: bool = False):
    key = (reps, debug_out)
    if key not in _NC_CACHE:
        _NC_CACHE[key] = build_program(reps, debug_out)
    return _NC_CACHE[key]


def kernel(x, w_attn, b_attn, w_proj, b_proj):
    x = np.asarray(x, np.float32)
    w_attn = np.asarray(w_attn, np.float32)
    b_attn = np.asarray(b_attn, np.float32)
    w_proj = np.asarray(w_proj, np.float32)
    b_proj = np.asarray(b_proj, np.float32)

    nc = get_program()
    in_maps = make_in_maps(x, w_attn, b_attn, w_proj)
    res = run_bass_kernel_spmd(nc, in_maps, core_ids=list(range(N_CORES)))
    acc = np.zeros((B, T, D), np.float64)
    for r in res.results:
        acc += r["out"].astype(np.float64)
    acc += host_bias(b_attn, b_proj, w_proj).astype(np.float64)
    return acc.astype(np.float32)


# revision 56
# speedup vs baseline: 1.0491x; 1.0066x over previous
"""Causal multi-head attention block (B=2, T=2048, D=1024, H=16) on 8 TRN2 cores.

Sharding: tensor-parallel over heads — each core owns 2 heads (128 cols of
w_attn's q/k/v blocks, 128 rows of w_proj) and produces a partial output
[B, T, D]; the host sums the 8 partials and adds the bias terms.

v5 layout (fp16 everywhere except the q/k projection, fp32 PSUM):
  - fp16 instead of bf16 (same PE/DVE rate, 8x the mantissa) keeps the
    base pipeline error at ~4e-4; the only deliberate precision spend is
    the q/k projection in fp8 e4m3 DoubleRow (pairs of d-tiles packed in
    the free dim, 2x PE rate; q/k feed softmax logits where ~1e-2 total
    error is tolerable). v / scores / AV / proj stay fp16.
  - k is stored zero-padded per head (kTz[h]: head h's rows + zeros) so
    score matmuls contract over the full 128 partitions with the
    two-head qT as moving operand: a 64-partition matmul measures ~2.4x
    slower per column on TRN2 hardware than a 128-partition one.
  - v is produced DIRECTLY in [t, hd] layout with x-stationary matmuls
    (stationary = x d-tile [128d, 128t], moving = w_v [128d, 128f]); the
    old vT + XBAR-transpose path (which stalled the PE ~11us) is gone.
    All four 128-t tiles of a chunk accumulate into one PSUM bank and
    are flipped into v_sb with a single strided DVE copy.
  - attention kt-loop is software-pipelined: scores/exp/mask of k-tile
    kt+1 are emitted BEFORE the AV matmuls of k-tile kt, so the
    scores->exp->AV round trip latency is hidden behind PE work instead
    of relying solely on filler items.
  - exp covers both heads in one ACT op (psum tile [128, 2, 512]); the
    causal mask is applied POST-exp as a {0,1} fp16 multiply on the
    diagonal 128-block of both heads in one cheap DVE op (4x DVE mode).
  - engine balance (measured on HW, ACT is the tightest): ACT = exp +
    q/k epilogues + one normalize staging copy; DVE = o-copies, masks,
    v epilogue, recip + normalize multiply; Pool = denominator
    broadcast + x DMA queue.
  - per-head normalize is emitted right after that head's last AV and
    stages av out of PSUM first, so the av bank frees immediately.
  - timing build (defer_tail) is fully rotated: batch-0 QKV of the NEXT
    pass and the last chunk's projections drain as fillers through this
    pass's attention, so the PE never sees a phase boundary.
  - PSUM budget (8 banks): scores 2x2 + av 2x1 + shared qkv/v/proj 2x1.
"""
import numpy as np

import concourse.bass as bass
import concourse.mybir as mybir
import concourse.tile as tile
from concourse import bacc
from concourse.bass import ts, ds
from concourse.bass_utils import run_bass_kernel_spmd

F32 = mybir.dt.float32
F16 = mybir.dt.float16
F8 = mybir.dt.float8e4
DR = mybir.MatmulPerfMode.DoubleRow

B, T, D = 2, 2048, 1024
H = 16
HD = D // H          # 64
N_CORES = 8
HPC = H // N_CORES   # heads per core = 2
CW = HPC * HD        # per-core head width = 128
TCH = 512            # q/t chunk width
NKT = T // 128       # 16 k-tiles per batch
NQC = T // TCH       # 4 q-chunks per batch
ND = D // 128        # 8 d-tiles
NT = B * T // 128    # 32 t-tiles total


def build_program(reps: int = 1, debug_out: bool = False):
    nc = bacc.Bacc("TRN2", target_bir_lowering=False, debug=False,
                   num_devices=N_CORES)

    xT = nc.dram_tensor("xT", [B, D, T], F16, kind="ExternalInput")
    x8T = nc.dram_tensor("x8T", [B, D, T], F8, kind="ExternalInput")
    wq = nc.dram_tensor("wq", [D, CW], F8, kind="ExternalInput")
    wk = nc.dram_tensor("wk", [D, CW], F8, kind="ExternalInput")
    wv = nc.dram_tensor("wv", [D, CW], F16, kind="ExternalInput")
    bq = nc.dram_tensor("bq", [CW], F32, kind="ExternalInput")   # pre-scaled /8
    bk = nc.dram_tensor("bk", [CW], F32, kind="ExternalInput")
    wp = nc.dram_tensor("wp", [CW, D], F16, kind="ExternalInput")
    # multiplicative causal mask for the diagonal 128-block, duplicated per
    # head: [128 k, HPC, 128 q], 1.0 where k <= q else 0.0
    mask = nc.dram_tensor("mask", [128, HPC, 128], F16, kind="ExternalInput")
    out = nc.dram_tensor("out", [B, T, D], F16, kind="ExternalOutput")
    if debug_out:
        dbg = {nm: nc.dram_tensor(f"dbg_{nm}", [128, B * T], F16,
                                  kind="ExternalOutput")
               for nm in ("qT", "kT", "a")}
        dbg["v2"] = nc.dram_tensor("dbg_v2", [128, NT, HPC, HD + 1],
                                   F16, kind="ExternalOutput")

    with tile.TileContext(nc) as tc:
        with (
            tc.tile_pool(name="const", bufs=1) as const,
            tc.tile_pool(name="persist", bufs=1) as persist,
            tc.tile_pool(name="xt", bufs=5) as xt_pool,
            tc.tile_pool(name="probs", bufs=6) as probs_pool,
            tc.tile_pool(name="norm", bufs=4) as norm_pool,
            tc.tile_pool(name="osb", bufs=8) as osb_pool,
        ):
            # ---- constants / persistent state ----
            wq_sb = const.tile([128, ND, CW], F8)
            wk_sb = const.tile([128, ND, CW], F8)
            wv_sb = const.tile([128, ND, CW], F16)
            wp_sb = const.tile([128, D], F16)
            bq_sb = const.tile([128, 1], F32)
            bk_sb = const.tile([128, 1], F32)
            mask_sb = const.tile([128, HPC, 128], F16)
            # consts on the sync queue; x streams on the gpsimd queue so the
            # steady-state x prefetch is not blocked behind out-writes.
            # wp/mask are not needed until attention starts — load them last.
            wq_r = wq.rearrange("(dt p) m -> p dt m", p=128)
            wk_r = wk.rearrange("(dt p) m -> p dt m", p=128)
            nc.sync.dma_start(wq_sb[:, 0:1, :], wq_r[:, 0:1, :])
            nc.sync.dma_start(wk_sb[:, 0:1, :], wk_r[:, 0:1, :])
            nc.sync.dma_start(wq_sb[:, 1:, :], wq_r[:, 1:, :])
            nc.sync.dma_start(wk_sb[:, 1:, :], wk_r[:, 1:, :])
            nc.sync.dma_start(wv_sb[:], wv.rearrange("(dt p) m -> p dt m", p=128))
            nc.sync.dma_start(bq_sb[:], bq[:, None])
            nc.sync.dma_start(bk_sb[:], bk[:, None])
            nc.sync.dma_start(mask_sb[:], mask[:, :, :])
            nc.sync.dma_start(wp_sb[:], wp[:, :])

            qT_sb = persist.tile([128, B * T], F16)   # [2h*64, (b,t)]
            # kT stored zero-padded per head: kTz[h] has head h's k in its
            # own 64 partition rows and ZEROS in the other head's rows, so
            # the score matmul contracts over the full 128 partitions (a
            # 64-partition matmul runs ~2.4x slower on the PE) with the
            # full two-head qT as the moving operand.
            kTz = [persist.tile([128, B * T], F16, name=f"kTz{h}")
                   for h in range(HPC)]
            nc.vector.memset(kTz[0][HD:128, :], 0.0)
            nc.vector.memset(kTz[1][0:HD, :], 0.0)
            a_sb = persist.tile([128, B * T], F16)    # normalized attn out ^T
            # v per t-tile & head: [ones | v(64)] columns — the LEADING ones
            # column makes the AV matmul emit the softmax denominator in
            # psum partition 0, where reciprocal/broadcast read it directly.
            v_sb = persist.tile([128, NT, HPC, HD + 1], F16)
            nc.vector.memset(v_sb[:, :, :, 0], 1.0)

            def body(defer_tail=False, _=None):
                with (
                    tc.tile_pool(name="ps_s", bufs=2, space="PSUM") as ps_s,
                    tc.tile_pool(name="ps_av", bufs=2, space="PSUM") as ps_av,
                    tc.tile_pool(name="ps_x", bufs=2, space="PSUM") as ps_x,
                ):
                    def emit_x_dma(c, fine=False):
                        """stream x chunk c: fp8 copy (q/k DoubleRow gemms)
                        first, then the fp16 copy (v gemm)."""
                        xt8 = xt_pool.tile([128, ND, TCH], F8, name=f"xt8{c}",
                                           tag="xt8")
                        xt = xt_pool.tile([128, ND, TCH], F16, name=f"xt{c}",
                                          tag="xt")
                        b_i, qc = divmod(c, NQC)
                        x8src = x8T[b_i].rearrange("(dt p) t -> p dt t",
                                                   p=128)[:, :, ds(qc * TCH, TCH)]
                        xsrc = xT[b_i].rearrange("(dt p) t -> p dt t", p=128)[
                            :, :, ds(qc * TCH, TCH)]
                        gw = 2 if fine and c <= 1 else ND
                        for g in range(ND // gw):
                            nc.gpsimd.dma_start(xt8[:, ts(g, gw), :],
                                                x8src[:, ts(g, gw), :])
                        for g in range(ND // gw):
                            nc.gpsimd.dma_start(xt[:, ts(g, gw), :],
                                                xsrc[:, ts(g, gw), :])
                        return xt8, xt

                    def qk_epi(kind, dst_sb, acc, ccol):
                        # on ACT (Identity with per-partition bias + scale):
                        # keeps the elastic psum drains off the DVE queue,
                        # which carries the latency-critical mask multiplies
                        if kind == "q":
                            nc.scalar.activation(
                                dst_sb[:, ccol], acc[:],
                                mybir.ActivationFunctionType.Identity,
                                bias=bq_sb[:], scale=0.125)
                        else:
                            # k lands in the zero-padded per-head tensors;
                            # both writes are lane-aligned (no shift).
                            # One half per engine to balance ACT/DVE load.
                            nc.scalar.activation(
                                kTz[0][0:HD, ccol], acc[0:HD, :],
                                mybir.ActivationFunctionType.Identity,
                                bias=bk_sb[0:HD])
                            nc.scalar.activation(
                                kTz[1][HD:128, ccol], acc[HD:128, :],
                                mybir.ActivationFunctionType.Identity,
                                bias=bk_sb[HD:128])

                    def emit_qk_mms(c, xt8):
                        """q/k projection via fp8 DoubleRow (pairs of d-tiles
                        packed in the free dim) + epilogues for chunk c."""
                        ccol = ds(c * TCH, TCH)
                        for dst_sb, w_t, kind in ((qT_sb, wq_sb, "q"),
                                                  (None, wk_sb, "k")):
                            acc = ps_x.tile([128, TCH], F32, tag="x", name="acc")
                            for p in range(ND // 2):
                                nc.tensor.matmul(acc[:],
                                                 w_t[:, ts(p, 2), :],
                                                 xt8[:, ts(p, 2), :],
                                                 start=p == 0,
                                                 stop=p == ND // 2 - 1,
                                                 perf_mode=DR)
                            qk_epi(kind, dst_sb, acc, ccol)

                    def emit_v_mms(c, xt):
                        """v for chunk c directly in [t, hd] layout:
                        stationary x t-tile, moving w_v. All 4 t-tiles
                        accumulate into one PSUM bank; one strided DVE copy
                        flips into v_sb [t, h, 1+hd]."""
                        vacc = ps_x.tile([128, TCH], F32, tag="x", name="vacc")
                        for tt in range(TCH // 128):
                            for dt in range(ND):
                                nc.tensor.matmul(
                                    vacc[:, ts(tt, 128)],
                                    xt[:, dt, ts(tt, 128)],
                                    wv_sb[:, dt, :],
                                    start=dt == 0, stop=dt == ND - 1)
                        gt0 = c * (TCH // 128)
                        nc.vector.tensor_copy(
                            v_sb[:, gt0:gt0 + 4, :, 1:HD + 1],
                            vacc.rearrange("p (tt h d) -> p tt h d",
                                           tt=4, h=HPC))

                    def emit_proj(pb, tt, ec, i):
                        o_ps = ps_x.tile([128, TCH], F32, tag="x", name="o")
                        nc.tensor.matmul(
                            o_ps[:],
                            a_sb[:, ds(pb * T + tt * 128, 128)],
                            wp_sb[:, ts(ec, TCH)],
                            start=True, stop=True)
                        o_sb = osb_pool.tile([128, TCH], F16)
                        nc.vector.tensor_copy(o_sb[:], o_ps[:])
                        nc.sync.dma_start(
                            out[pb, ts(tt, 128), ts(ec, TCH)], o_sb[:])

                    # ---- unified PE filler queue (see v4) ----
                    fillq = []   # [(need_before_attn_chunk, fn)]

                    def qkv_items(c, xt8, xt):
                        """filler items for chunk c's q/k/v matmuls."""
                        items = []
                        ccol = ds(c * TCH, TCH)
                        for dst_sb, w_t, kind in ((qT_sb, wq_sb, "q"),
                                                  (None, wk_sb, "k")):
                            box = []

                            def h1(box=box, w_t=w_t, xt8=xt8):
                                acc = ps_x.tile([128, TCH], F32, tag="x",
                                                name="acc")
                                box.append(acc)
                                for p in range(ND // 4):
                                    nc.tensor.matmul(
                                        acc[:], w_t[:, ts(p, 2), :],
                                        xt8[:, ts(p, 2), :],
                                        start=p == 0, stop=False,
                                        perf_mode=DR)

                            def h2(box=box, w_t=w_t, xt8=xt8, kind=kind,
                                   dst_sb=dst_sb, ccol=ccol):
                                acc = box[0]
                                for p in range(ND // 4, ND // 2):
                                    nc.tensor.matmul(
                                        acc[:], w_t[:, ts(p, 2), :],
                                        xt8[:, ts(p, 2), :],
                                        start=False, stop=p == ND // 2 - 1,
                                        perf_mode=DR)
                                qk_epi(kind, dst_sb, acc, ccol)

                            items += [h1, h2]

                        vbox = []

                        def v_item(tt, c=c, xt=xt, vbox=vbox):
                            if tt == 0:
                                vacc = ps_x.tile([128, TCH], F32, tag="x",
                                                 name="vacc")
                                vbox.append(vacc)
                            vacc = vbox[0]
                            for dt in range(ND):
                                nc.tensor.matmul(
                                    vacc[:, ts(tt, 128)],
                                    xt[:, dt, ts(tt, 128)],
                                    wv_sb[:, dt, :],
                                    start=dt == 0, stop=dt == ND - 1)
                            if tt == 3:
                                gt0 = c * (TCH // 128)
                                nc.vector.tensor_copy(
                                    v_sb[:, gt0:gt0 + 4, :, 1:HD + 1],
                                    vacc.rearrange("p (tt h d) -> p tt h d",
                                                   tt=4, h=HPC))

                        items += [lambda tt=tt: v_item(tt) for tt in range(4)]
                        return items

                    # ---- phase 1a: QKV for batch 0 ----
                    # In the rotated steady-state body (defer_tail), batch
                    # 0's QKV for the NEXT pass drains as fillers through
                    # batch 1's attention; this pass's batch-0 attention
                    # reads qT/kT/v written by the previous pass.
                    if not defer_tail:
                        xts = {c: emit_x_dma(c, fine=True) for c in range(2)}
                        for c in range(NQC):
                            if c + 2 < NQC:
                                xts[c + 2] = emit_x_dma(c + 2)
                            xt8, xt = xts.pop(c)
                            emit_qk_mms(c, xt8)
                            emit_v_mms(c, xt)

                    # ---- attention; QKV for batch 1 + all projections are
                    # drained through the filler queue ----
                    if defer_tail:
                        fillq += [
                            (7, lambda t_=t_, e_=e_: emit_proj(
                                B - 1, t_, e_, t_ * 2 + e_))
                            for t_ in range((NQC - 1) * 4, NQC * 4)
                            for e_ in range(D // TCH)]
                    xts_n = {}
                    for b_i in range(B):
                        for qc in range(NQC):
                            g = b_i * NQC + qc   # global chunk index
                            if b_i == 0:
                                xt_c = emit_x_dma(4 + qc)
                                fillq += [(4 + qc, fn)
                                          for fn in qkv_items(4 + qc, *xt_c)]
                                if defer_tail and qc >= 2:
                                    # rotation: prefetch next pass's first
                                    # batch-0 x chunks early
                                    xts_n[qc - 2] = emit_x_dma(qc - 2)
                            elif defer_tail:
                                # rotation: next pass's batch-0 QKV drains
                                # as fillers through batch-1 attention
                                if qc < 2:
                                    xts_n[qc + 2] = emit_x_dma(qc + 2)
                                if qc == 0:
                                    for c_n in (0, 1):
                                        fillq += [(99, fn) for fn in
                                                  qkv_items(c_n, *xts_n[c_n])]
                                elif qc == 1:
                                    for c_n in (2, 3):
                                        fillq += [(99, fn) for fn in
                                                  qkv_items(c_n, *xts_n[c_n])]
                            # force-drain items this chunk depends on
                            rest = []
                            for need, fn in fillq:
                                if need <= g:
                                    fn()
                                else:
                                    rest.append((need, fn))
                            fillq = rest
                            qcol = ds(b_i * T + qc * TCH, TCH)
                            nkt = 4 * qc + 4   # causal: k-tiles 0..nkt-1
                            avs = [ps_av.tile([HD + 1, TCH], F32, tag="av",
                                              name=f"av{_h}")
                                   for _h in range(HPC)]

                            def emit_scores(kt):
                                """scores + exp + mask for k-tile kt; returns
                                (pp tile, fsl) ready for the AV matmuls."""
                                j = kt - 4 * qc   # >=0: straddles diagonal
                                f0 = max(j, 0) * 128
                                fsl = ds(f0, TCH - f0)
                                ktcol = ds(b_i * T + kt * 128, 128)
                                sp = ps_s.tile([128, HPC, TCH], F32,
                                               tag="sp", name="sp")
                                pp = probs_pool.tile(
                                    [128, HPC, TCH], F16,
                                    tag="pp", name="pp")
                                for h in range(HPC):
                                    nc.tensor.matmul(
                                        sp[:, h, fsl],
                                        kTz[h][:, ktcol],
                                        qT_sb[:, qcol][:, fsl],
                                        start=True, stop=True)
                                # one exp covers both heads (fewer ACT ops;
                                # a per-head split measured 8us slower, and
                                # Schraudolph exp-on-DVE measured 6us slower
                                # at a 1/3 fraction: both queues are tight)
                                if f0 == 0:
                                    nc.scalar.activation(
                                        pp.rearrange("p h w -> p (h w)"),
                                        sp.rearrange("p h w -> p (h w)"),
                                        mybir.ActivationFunctionType.Exp)
                                elif f0 == 0:
                                    nc.scalar.activation(
                                        pp.rearrange("p h w -> p (h w)"),
                                        sp.rearrange("p h w -> p (h w)"),
                                        mybir.ActivationFunctionType.Exp)
                                else:
                                    nc.scalar.activation(
                                        pp[:, :, fsl], sp[:, :, fsl],
                                        mybir.ActivationFunctionType.Exp)
                                if j >= 0:
                                    # multiplicative causal mask on the
                                    # diagonal 128-block of both heads
                                    dsl = ds(f0, 128)
                                    nc.vector.tensor_tensor(
                                        pp[:, :, dsl], pp[:, :, dsl],
                                        mask_sb[:, :, :],
                                        mybir.AluOpType.mult)
                                return pp, fsl

                            def emit_norm(h):
                                """normalize rows 1..64 by denominator row 0;
                                emitted immediately after head h's last AV.
                                The av psum bank is freed by a single staging
                                copy (one ACT/DVE op) so the next chunk's AV
                                never waits on the recip/bcast/mult chain."""
                                hp = ds(h * HD, HD)
                                av_ps = avs[h]
                                av_s = norm_pool.tile([HD + 1, TCH], F32,
                                                      tag=f"avs{h}")
                                if h == 0:
                                    nc.scalar.copy(av_s[:], av_ps[:])
                                else:
                                    nc.vector.tensor_copy(av_s[:], av_ps[:])
                                r0_sb = norm_pool.tile([1, TCH], F32,
                                                       tag="r0")
                                nc.vector.reciprocal_approx_fast(
                                    r0_sb[:], av_ps[0:1, :])
                                bc_sb = norm_pool.tile([HD + 1, TCH], F32,
                                                       tag="bc")
                                nc.gpsimd.partition_broadcast(
                                    bc_sb[:], r0_sb[:])
                                at_sb = norm_pool.tile([HD + 1, TCH], F16,
                                                       tag="at")
                                # row 0 computes junk (denom*recip) that no
                                # reader touches; base must be 0 (aligned)
                                nc.vector.tensor_tensor(
                                    at_sb[0:HD + 1, :], av_s[0:HD + 1, :],
                                    bc_sb[0:HD + 1, :],
                                    mybir.AluOpType.mult)
                                # partition-shift into stacked-head layout
                                nc.sync.dma_start(a_sb[hp, qcol],
                                                  at_sb[1:HD + 1, :])

                            def emit_av(kt, pp, fsl):
                                for h in range(HPC):
                                    nc.tensor.matmul(
                                        avs[h][:, fsl],
                                        v_sb[:, b_i * NKT + kt, h, :],
                                        pp[:, h, fsl],
                                        start=kt == 0, stop=kt == nkt - 1)
                                    if kt == nkt - 1:
                                        emit_norm(h)

                            # software pipeline: AV lags scores by one k-tile
                            pend = None
                            for kt in range(nkt):
                                cur = (kt, *emit_scores(kt))
                                if pend is not None:
                                    npop = 2 if len(fillq) > 10 else 1
                                    for _p in range(min(npop, len(fillq))):
                                        fillq.pop(0)[1]()
                                    emit_av(*pend)
                                pend = cur
                            if fillq:
                                fillq.pop(0)[1]()
                            emit_av(*pend)

                            items = [(b_i, t_, e_, t_ * 2 + e_)
                                     for t_ in range(qc * 4, qc * 4 + 4)
                                     for e_ in range(D // TCH)]
                            last = (b_i == B - 1 and qc == NQC - 1)
                            if not last:
                                fillq += [(99, lambda a=a: emit_proj(*a))
                                          for a in items]
                            elif not defer_tail:
                                for a in items:
                                    emit_proj(*a)
                            if last:
                                for _, fn in fillq:
                                    fn()
                                fillq = []

            if reps < 0:
                # static unroll of the rotated body, for TimelineSim only
                # (the sim cannot follow dynamic loops)
                for _r in range(-reps):
                    body(defer_tail=True)
            elif reps == 1:
                body(defer_tail=False)
                if debug_out:
                    nc.sync.dma_start(dbg["qT"][:, :], qT_sb[:])
                    nc.sync.dma_start(dbg["kT"][:, :], kTz[0][:])
                    nc.sync.dma_start(dbg["a"][:, :], a_sb[:])
                    nc.sync.dma_start(dbg["v2"][:, :, :, :], v_sb[:])
            else:
                with tc.For_i(0, reps, 1) as _i:
                    body(defer_tail=True, _=_i)

    nc.compile()
    return nc


def make_mask() -> np.ndarray:
    """Multiplicative causal mask for the 128x128 diagonal block, duplicated
    per head: keeps (p <= f)."""
    p = np.arange(128)[:, None]
    f = np.arange(128)[None, :]
    m = (p <= f).astype(np.float16)
    return np.broadcast_to(m[:, None, :], (128, HPC, 128)).copy()


def _e4m3(a):
    import ml_dtypes
    return np.clip(a, -240.0, 240.0).astype(ml_dtypes.float8_e4m3fn)


def make_in_maps(x, w_attn, b_attn, w_proj):
    f16 = np.float16
    xT = np.ascontiguousarray(np.transpose(x, (0, 2, 1))).astype(f16)
    x8T = _e4m3(xT.astype(np.float32))
    mask = make_mask()
    in_maps = []
    for c in range(N_CORES):
        cs = slice(CW * c, CW * (c + 1))
        in_maps.append({
            "xT": xT,
            "x8T": x8T,
            "wq": _e4m3(np.ascontiguousarray(w_attn[:, 0 * D:1 * D][:, cs])),
            "wk": _e4m3(np.ascontiguousarray(w_attn[:, 1 * D:2 * D][:, cs])),
            "wv": np.ascontiguousarray(w_attn[:, 2 * D:3 * D][:, cs]).astype(f16),
            "bq": np.ascontiguousarray(b_attn[0 * D:1 * D][cs]) * 0.125,
            "bk": np.ascontiguousarray(b_attn[1 * D:2 * D][cs]),
            "wp": np.ascontiguousarray(w_proj[cs, :]).astype(f16),
            "mask": mask,
        })
    return in_maps


def host_bias(b_attn, b_proj, w_proj):
    # v-bias propagates exactly through softmax (rows sum to 1) and the linear
    # projection: out += b_v @ w_proj + b_proj
    return b_proj.astype(np.float32) + b_attn[2 * D:3 * D].astype(np.float32) @ w_proj.astype(np.float32)


_NC_CACHE = {}


def get_program(reps: int = 1, debug_out: bool = False):
    key = (reps, debug_out)
    if key not in _NC_CACHE:
        _NC_CACHE[key] = build_program(reps, debug_out)
    return _NC_CACHE[key]


def kernel(x, w_attn, b_attn, w_proj, b_proj):
    x = np.asarray(x, np.float32)
    w_attn = np.asarray(w_attn, np.float32)
    b_attn = np.asarray(b_attn, np.float32)
    w_proj = np.asarray(w_proj, np.float32)
    b_proj = np.asarray(b_proj, np.float32)

    nc = get_program()
    in_maps = make_in_maps(x, w_attn, b_attn, w_proj)
    res = run_bass_kernel_spmd(nc, in_maps, core_ids=list(range(N_CORES)))
    acc = np.zeros((B, T, D), np.float64)
    for r in res.results:
        acc += r["out"].astype(np.float64)
    acc += host_bias(b_attn, b_proj, w_proj).astype(np.float64)
    return acc.astype(np.float32)
